# revision 8
# baseline (speedup 1.0000x reference)
"""Grouped-experts GEMM (MoE ragged dot) on 8 TRN2 NeuronCores.

Sharding: tensor-parallel over out_features. Every core sees all tokens
(identical SPMD program) and computes a disjoint 512-wide slice of the
4096 output columns; gather is a host-side concatenate, no collectives.

Compute: mixed bf16 + fp8 (e4m3) with fp32 PSUM accumulation. A fraction
of the K=2048 contraction runs as fp8 DoubleRow pairs (2 fp8 MACs per PE
cell per cycle): the first-processed (largest) expert runs 12 of 16
K-tiles in fp8, the rest run 2 of 16. The 2e-2 rel-err budget is spent
where it buys the most compute (measured HW rel err 1.863e-2, matching
the host e4m3 simulation exactly). fp8 operands are pre-scaled by
SX=8/SW=128 to clear the e4m3 subnormal region; the bf16 W carries the
combined 1024x so one exact tensor_scalar multiply (1/1024) in the
output copy undoes it. Outputs store as bf16 (host upcasts).

Schedule (trace-driven):
 - Experts processed in descending padded-tile order, so each later 2MB
   W load always trails the X stream with slack.
 - Warm-up dummy matmuls before and DMA-paced inside m-tile 0 keep the
   PE busy through the DMA-bound prologue, holding the HAM clock gate at
   K=8/8 (2.4 GHz) instead of idling cold at 1.2 GHz.
 - The first expert's W + first X chunk load as interleaved k-pair
   pieces so the first matmul waits on ~0.4MB, not 3MB.
 - start=True on a sub-bank matmul clears has_written for the WHOLE
   PSUM bank on HW: only the first matmul of each tile's group sets it.

fp8 pairs run as ONE full-width DoubleRow matmul each (fp8 moving
operand supports 128x1024): ~228ns/pair vs 309ns for two 256-wide halves.

Measured: ~219us HW exec (baseline bf16 kernel: ~236us); bf16 floor
for this shape would be ~221us of pure matmul issue alone.

Fallback: if the first expert's padded tile count is odd (chunks would
straddle the fp8-class boundary), everything runs k8=2 uniformly.
"""

import sys

import numpy as np

sys.path.insert(0, "/opt/trn_rl_repo")

import ml_dtypes

NUM_TOKENS = 8192
IN_FEATURES = 2048
OUT_FEATURES = 4096
GROUPS = 8
N_CORES = 8

P = 128
KT = IN_FEATURES // P  # 16 K-tiles of 128
N_CORE = OUT_FEATURES // N_CORES  # 512 output cols per core

K8A = 12  # fp8 K-tiles for the first (largest) expert
K8B = 2  # fp8 K-tiles for the rest

SX = 8.0  # fp8 x pre-scale
SW = 128.0  # fp8 w pre-scale
SCALE = SX * SW  # PSUM carries SCALE x true output
PRE_WARM = 8
DR_FILL = 2
PAIR_FILL = 3

_BUILD_CACHE: dict = {}


def _build_program(units: tuple[int, ...], k8a: int):
    """SPMD program for per-position padded M-tile counts `units`
    (processing order). Position 0 runs k8a fp8 K-tiles, rest K8B."""
    import concourse.mybir as mybir
    import concourse.tile as tile
    from concourse import bacc

    f32 = mybir.dt.float32
    bf16 = mybir.dt.bfloat16
    f8 = mybir.dt.float8e4
    DR = mybir.MatmulPerfMode.DoubleRow
    U = sum(units)
    assert U % 2 == 0
    has_a = k8a != K8B  # no class A in the fallback (odd first-expert units)
    UA = units[0] if has_a else 0
    assert UA % 2 == 0
    CA = UA // 2  # class-A chunks
    CB = (U - UA) // 2
    KBA, KBB = KT - k8a, KT - K8B
    GB = sum(1 for u in units[1 if has_a else 0 :] if u > 0)

    nc = bacc.Bacc(None, target_bir_lowering=False)
    dp = nc.declare_dram_parameter
    xa_p = dp("xa", [max(CA, 1), P, KBA, 2 * P], bf16, isOutput=False)
    x8a_p = dp("x8a", [max(CA, 1), P, k8a, 2 * P], f8, isOutput=False)
    wa_p = dp("wa", [P, KBA, N_CORE], bf16, isOutput=False)
    w8a_p = dp("w8a", [P, k8a, N_CORE], f8, isOutput=False)
    xb_p = dp("xb", [max(CB, 1), P, KBB, 2 * P], bf16, isOutput=False)
    x8b_p = dp("x8b", [max(CB, 1), P, K8B, 2 * P], f8, isOutput=False)
    wb_p = dp("wb", [max(GB, 1), P, KBB, N_CORE], bf16, isOutput=False)
    w8b_p = dp("w8b", [max(GB, 1), P, K8B, N_CORE], f8, isOutput=False)
    o_p = dp("out", [U * P, N_CORE], bf16, isOutput=True)

    with tile.TileContext(nc) as tc:
        with (
            tc.tile_pool(name="warm", bufs=1) as warmpool,
            tc.tile_pool(name="xp", bufs=4) as xpool,
            tc.tile_pool(name="x8p", bufs=4) as x8pool,
            tc.tile_pool(name="wp", bufs=3) as wpool,
            tc.tile_pool(name="w8p", bufs=3) as w8pool,
            tc.tile_pool(name="op", bufs=4) as opool,
            tc.tile_pool(name="ps", bufs=4, space="PSUM") as pspool,
        ):
            owner = []
            for e in range(len(units)):
                owner += [e] * units[e]

            scratch = warmpool.tile([P, P + N_CORE], bf16, tag="warm")
            nc.vector.memset(scratch[:], 0.0)
            wps = pspool.tile([P, N_CORE], f32, tag="ps")

            def dummy(n):
                for _ in range(n):
                    nc.tensor.matmul(
                        wps[:], scratch[:, :P], scratch[:, P:],
                        start=True, stop=True, skip_group_check=True,
                    )

            dummy(PRE_WARM)

            def cls_of(e):  # (k8t, kbt, x8_p, x_p, w8_ap, w_ap, cbase)
                if e == 0 and has_a:
                    return (k8a, KBA, x8a_p, xa_p, w8a_p, wa_p, 0)
                eb = e - 1 if has_a else e
                return (K8B, KBB, x8b_p, xb_p, w8b_p[eb], wb_p[eb], CA)

            # Prologue: first expert's W + first chunk, k-pair pieces,
            # fp8 pairs first.
            e0 = owner[0]
            k8t0, kbt0, x8s, xs, w8s, ws, _ = cls_of(e0)
            w8_0 = w8pool.tile([P, k8t0, N_CORE], f8, tag="w8")
            x8_0 = x8pool.tile([P, k8t0, 2 * P], f8, tag="x8")
            w0 = wpool.tile([P, kbt0, N_CORE], bf16, tag="w")
            x0 = xpool.tile([P, kbt0, 2 * P], bf16, tag="x")
            for kp in range(k8t0 // 2):
                s = slice(2 * kp, 2 * kp + 2)
                nc.sync.dma_start(out=x8_0[:, s], in_=x8s[0, :, s])
                nc.sync.dma_start(out=w8_0[:, s], in_=w8s[:, s])
            for kp in range(kbt0 // 2):
                s = slice(2 * kp, 2 * kp + 2)
                nc.sync.dma_start(out=x0[:, s], in_=xs[0, :, s])
                nc.sync.dma_start(out=w0[:, s], in_=ws[:, s])

            w_cur = (e0, w0, w8_0)
            x_cur = (0, x0, x8_0)
            for m in range(U):
                e = owner[m]
                k8t, kbt, x8s, xs, w8s, ws, cbase = cls_of(e)
                if w_cur[0] != e:
                    w8_t = w8pool.tile([P, k8t, N_CORE], f8, tag="w8")
                    nc.sync.dma_start(out=w8_t[:], in_=w8s[:])
                    w_t = wpool.tile([P, kbt, N_CORE], bf16, tag="w")
                    nc.sync.dma_start(out=w_t[:], in_=ws[:])
                    w_cur = (e, w_t, w8_t)
                _, w_t, w8_t = w_cur
                c, half = divmod(m, 2)
                if x_cur[0] != c:
                    x8_t = x8pool.tile([P, k8t, 2 * P], f8, tag="x8")
                    nc.sync.dma_start(out=x8_t[:], in_=x8s[c - cbase])
                    x_t = xpool.tile([P, kbt, 2 * P], bf16, tag="x")
                    nc.sync.dma_start(out=x_t[:], in_=xs[c - cbase])
                    x_cur = (c, x_t, x8_t)
                _, x_t, x8_t = x_cur

                hs = slice(half * P, (half + 1) * P)
                ps = pspool.tile([P, N_CORE], f32, tag="ps")
                # fp8 DoubleRow pairs, one full-width matmul per pair
                # (fp8 moving operand supports 128x1024).
                for j in range(k8t // 2):
                    js = slice(2 * j, 2 * j + 2)
                    nc.tensor.matmul(
                        ps[:],
                        x8_t[:, js, hs],
                        w8_t[:, js, :],
                        start=(j == 0),
                        stop=False,
                        perf_mode=DR,
                        skip_group_check=True,
                    )
                    if m == 0:
                        dummy(DR_FILL)
                for k in range(kbt):
                    nc.tensor.matmul(
                        ps[:],
                        x_t[:, k, hs],
                        w_t[:, k, :],
                        start=False,
                        stop=(k == kbt - 1),
                        skip_group_check=True,
                    )
                    if m == 0 and k % 2 == 1:
                        dummy(PAIR_FILL)
                o_t = opool.tile([P, N_CORE], bf16, tag="o")
                nc.vector.tensor_scalar_mul(o_t[:], ps[:], 1.0 / SCALE)
                nc.scalar.dma_start(out=o_p[m * P : (m + 1) * P, :], in_=o_t[:])
    nc.compile()
    return nc


def _get_program(units: tuple[int, ...], k8a: int):
    key = (units, k8a)
    if key not in _BUILD_CACHE:
        _BUILD_CACHE[key] = _build_program(units, k8a)
    return _BUILD_CACHE[key]


def _segments(tokens_per_expert: np.ndarray, total: int):
    sizes = []
    start = 0
    for e in range(GROUPS):
        s = int(max(0, tokens_per_expert[e]))
        s = min(s, total - start)
        sizes.append(s)
        start += s
    return sizes


def _pack(Xp, k8t, f8s, bfs):
    """Xp [R, K] fp32 -> (x8 [C, P, k8t, 2P] e4m3, xb [C, P, KT-k8t, 2P] bf16)."""
    R, K = Xp.shape
    C = R // (2 * P)
    K8 = k8t * P
    x8 = np.ascontiguousarray(
        (Xp[:, :K8] * f8s).astype(ml_dtypes.float8_e4m3)
        .reshape(C, 2 * P, k8t, P).transpose(0, 3, 2, 1)
    )
    xb = np.ascontiguousarray(
        (Xp[:, K8:] * bfs).astype(ml_dtypes.bfloat16)
        .reshape(C, 2 * P, KT - k8t, P).transpose(0, 3, 2, 1)
    )
    return x8, xb


def kernel(input, weight, tokens_per_expert, _trace=False, _trace_kwargs=None):
    from concourse.bass_utils import run_bass_kernel_spmd

    x = np.asarray(input, dtype=np.float32)
    w = np.asarray(weight, dtype=np.float32)
    tpe = np.asarray(tokens_per_expert, dtype=np.int64)
    T, K = x.shape
    G, K2, N = w.shape
    assert (T, K, G, K2, N) == (NUM_TOKENS, IN_FEATURES, GROUPS, IN_FEATURES, OUT_FEATURES)

    sizes = _segments(tpe, T)
    units = [-(-s // P) for s in sizes]
    if sum(units) % 2:
        for e in range(GROUPS):
            if units[e] > 0 or e == GROUPS - 1:
                units[e] += 1
                break
    out = np.zeros((T, N), dtype=np.float32)
    U = sum(units)
    if U == 0:
        return out

    proc = sorted(range(GROUPS), key=lambda e: (-units[e], e))
    units_proc = tuple(units[e] for e in proc)
    k8a = K8A if units_proc[0] % 2 == 0 and units_proc[0] > 0 else K8B
    mstarts_proc = np.concatenate([[0], np.cumsum(units_proc)])[:GROUPS]
    pos = [0] * GROUPS
    for i, e in enumerate(proc):
        pos[e] = i
    orig_starts = np.concatenate([[0], np.cumsum(sizes)])[:GROUPS]

    Xp = np.zeros((U * P, K), dtype=np.float32)
    for e in range(GROUPS):
        s = sizes[e]
        if s:
            r0 = mstarts_proc[pos[e]] * P
            Xp[r0 : r0 + s] = x[orig_starts[e] : orig_starts[e] + s]

    has_a = k8a != K8B
    UA = units_proc[0] if has_a else 0
    RA = UA * P
    if has_a:
        x8a, xa = _pack(Xp[:RA], k8a, SX, 1.0)
    else:
        x8a = np.zeros((1, P, k8a, 2 * P), ml_dtypes.float8_e4m3)
        xa = np.zeros((1, P, KT - k8a, 2 * P), ml_dtypes.bfloat16)
    if U > UA:
        x8b, xb = _pack(Xp[RA:], K8B, SX, 1.0)
    else:
        x8b = np.zeros((1, P, K8B, 2 * P), ml_dtypes.float8_e4m3)
        xb = np.zeros((1, P, KT - K8B, 2 * P), ml_dtypes.bfloat16)

    wproc = w[proc]  # [G, K, N] processing order
    K8a = k8a * P
    K8b = K8B * P

    def wpack(mat, k8, scale8, scaleb, cslice):
        w8 = np.ascontiguousarray(
            (mat[:k8 * P, cslice] * scale8).astype(ml_dtypes.float8_e4m3)
            .reshape(k8, P, -1).transpose(1, 0, 2)
        )
        wb = np.ascontiguousarray(
            (mat[k8 * P :, cslice] * scaleb).astype(ml_dtypes.bfloat16)
            .reshape(KT - k8, P, -1).transpose(1, 0, 2)
        )
        return w8, wb

    boff = 1 if has_a else 0
    GB = sum(1 for u in units_proc[boff:] if u > 0)
    in_maps = []
    for cidx in range(N_CORES):
        cs = slice(cidx * N_CORE, (cidx + 1) * N_CORE)
        w8a_d, wa_d = wpack(wproc[0], k8a, SW, SCALE, cs)
        if GB:
            pk = [wpack(wproc[boff + i], K8B, SW, SCALE, cs) for i in range(GB)]
            w8b_d = np.stack([p[0] for p in pk])
            wb_d = np.stack([p[1] for p in pk])
        else:
            w8b_d = np.zeros((1, P, K8B, N_CORE), ml_dtypes.float8_e4m3)
            wb_d = np.zeros((1, P, KT - K8B, N_CORE), ml_dtypes.bfloat16)
        in_maps.append(
            {"xa": xa, "x8a": x8a, "xb": xb, "x8b": x8b,
             "wa": wa_d, "w8a": w8a_d, "wb": wb_d, "w8b": w8b_d}
        )

    nc = _get_program(units_proc, k8a)
    kw = dict(_trace_kwargs or {})
    res = run_bass_kernel_spmd(nc, in_maps, list(range(N_CORES)), trace=_trace, **kw)
    full = np.concatenate(
        [np.asarray(res.results[c]["out"]).astype(np.float32) for c in range(N_CORES)],
        axis=1,
    )

    for e in range(GROUPS):
        s = sizes[e]
        if s:
            r0 = mstarts_proc[pos[e]] * P
            out[orig_starts[e] : orig_starts[e] + s] = full[r0 : r0 + s]
    if _trace:
        return out, res
    return out


# revision 9
# speedup vs baseline: 1.0048x; 1.0048x over previous
"""Grouped-experts GEMM (MoE ragged dot) on 8 TRN2 NeuronCores.

Sharding: tensor-parallel over out_features. Every core sees all tokens
(identical SPMD program) and computes a disjoint 512-wide slice of the
4096 output columns; gather is a host-side concatenate, no collectives.

Compute: mixed bf16 + fp8 (e4m3) with fp32 PSUM accumulation. A fraction
of the K=2048 contraction runs as fp8 DoubleRow pairs (2 fp8 MACs per PE
cell per cycle): the first-processed (largest) expert runs 12 of 16
K-tiles in fp8, the rest run 2 of 16. The 2e-2 rel-err budget is spent
where it buys the most compute (measured HW rel err 1.863e-2, matching
the host e4m3 simulation exactly). fp8 operands are pre-scaled by
SX=8/SW=128 to clear the e4m3 subnormal region; the bf16 W carries the
combined 1024x so one exact tensor_scalar multiply (1/1024) in the
output copy undoes it. Outputs store as bf16 (host upcasts).

Schedule (trace-driven):
 - Experts processed in descending padded-tile order, so each later 2MB
   W load always trails the X stream with slack.
 - Warm-up dummy matmuls before and DMA-paced inside m-tile 0 keep the
   PE busy through the DMA-bound prologue, holding the HAM clock gate at
   K=8/8 (2.4 GHz) instead of idling cold at 1.2 GHz.
 - The first expert's W + first X chunk load as interleaved k-pair
   pieces so the first matmul waits on ~0.4MB, not 3MB.
 - start=True on a sub-bank matmul clears has_written for the WHOLE
   PSUM bank on HW: only the first matmul of each tile's group sets it.

fp8 pairs run as ONE full-width DoubleRow matmul each (fp8 moving
operand supports 128x1024): ~228ns/pair vs 309ns for two 256-wide halves.

Measured: ~219us HW exec (baseline bf16 kernel: ~236us); bf16 floor
for this shape would be ~221us of pure matmul issue alone.

Fallback: if the first expert's padded tile count is odd (chunks would
straddle the fp8-class boundary), everything runs k8=2 uniformly.
"""

import sys

import numpy as np

sys.path.insert(0, "/opt/trn_rl_repo")

import ml_dtypes

NUM_TOKENS = 8192
IN_FEATURES = 2048
OUT_FEATURES = 4096
GROUPS = 8
N_CORES = 8

P = 128
KT = IN_FEATURES // P  # 16 K-tiles of 128
N_CORE = OUT_FEATURES // N_CORES  # 512 output cols per core

K8A = 12  # fp8 K-tiles for the first (largest) expert
K8B = 2  # fp8 K-tiles for the rest

SX = 8.0  # fp8 x pre-scale
SW = 128.0  # fp8 w pre-scale
SCALE = SX * SW  # PSUM carries SCALE x true output
PRE_WARM = 8
DR_FILL = 2
PAIR_FILL = 3

_BUILD_CACHE: dict = {}


def _build_program(units: tuple[int, ...], k8a: int):
    """SPMD program for per-position padded M-tile counts `units`
    (processing order). Position 0 runs k8a fp8 K-tiles, rest K8B."""
    import concourse.mybir as mybir
    import concourse.tile as tile
    from concourse import bacc

    f32 = mybir.dt.float32
    bf16 = mybir.dt.bfloat16
    f8 = mybir.dt.float8e4
    DR = mybir.MatmulPerfMode.DoubleRow
    U = sum(units)
    assert U % 2 == 0
    has_a = k8a != K8B  # no class A in the fallback (odd first-expert units)
    UA = units[0] if has_a else 0
    assert UA % 2 == 0
    CA = UA // 2  # class-A chunks
    CB = (U - UA) // 2
    KBA, KBB = KT - k8a, KT - K8B
    GB = sum(1 for u in units[1 if has_a else 0 :] if u > 0)

    nc = bacc.Bacc(None, target_bir_lowering=False)
    dp = nc.declare_dram_parameter
    xa_p = dp("xa", [max(CA, 1), P, KBA, 2 * P], bf16, isOutput=False)
    x8a_p = dp("x8a", [max(CA, 1), P, k8a, 2 * P], f8, isOutput=False)
    wa_p = dp("wa", [P, KBA, N_CORE], bf16, isOutput=False)
    w8a_p = dp("w8a", [P, k8a, N_CORE], f8, isOutput=False)
    xb_p = dp("xb", [max(CB, 1), P, KBB, 2 * P], bf16, isOutput=False)
    x8b_p = dp("x8b", [max(CB, 1), P, K8B, 2 * P], f8, isOutput=False)
    wb_p = dp("wb", [max(GB, 1), P, KBB, N_CORE], bf16, isOutput=False)
    w8b_p = dp("w8b", [max(GB, 1), P, K8B, N_CORE], f8, isOutput=False)
    o_p = dp("out", [U * P, N_CORE], bf16, isOutput=True)

    with tile.TileContext(nc) as tc:
        with (
            tc.tile_pool(name="warm", bufs=1) as warmpool,
            tc.tile_pool(name="xp", bufs=5) as xpool,
            tc.tile_pool(name="x8p", bufs=5) as x8pool,
            tc.tile_pool(name="wp", bufs=3) as wpool,
            tc.tile_pool(name="w8p", bufs=3) as w8pool,
            tc.tile_pool(name="op", bufs=4) as opool,
            tc.tile_pool(name="ps", bufs=4, space="PSUM") as pspool,
        ):
            owner = []
            for e in range(len(units)):
                owner += [e] * units[e]

            scratch = warmpool.tile([P, P + N_CORE], bf16, tag="warm")
            nc.vector.memset(scratch[:], 0.0)
            wps = pspool.tile([P, N_CORE], f32, tag="ps")

            def dummy(n):
                for _ in range(n):
                    nc.tensor.matmul(
                        wps[:], scratch[:, :P], scratch[:, P:],
                        start=True, stop=True, skip_group_check=True,
                    )

            dummy(PRE_WARM)

            def cls_of(e):  # (k8t, kbt, x8_p, x_p, w8_ap, w_ap, cbase)
                if e == 0 and has_a:
                    return (k8a, KBA, x8a_p, xa_p, w8a_p, wa_p, 0)
                eb = e - 1 if has_a else e
                return (K8B, KBB, x8b_p, xb_p, w8b_p[eb], wb_p[eb], CA)

            # Prologue: first expert's W + first chunk, k-pair pieces,
            # fp8 pairs first.
            e0 = owner[0]
            k8t0, kbt0, x8s, xs, w8s, ws, _ = cls_of(e0)
            w8_0 = w8pool.tile([P, k8t0, N_CORE], f8, tag="w8")
            x8_0 = x8pool.tile([P, k8t0, 2 * P], f8, tag="x8")
            w0 = wpool.tile([P, kbt0, N_CORE], bf16, tag="w")
            x0 = xpool.tile([P, kbt0, 2 * P], bf16, tag="x")
            for kp in range(k8t0 // 2):
                s = slice(2 * kp, 2 * kp + 2)
                nc.sync.dma_start(out=x8_0[:, s], in_=x8s[0, :, s])
                nc.sync.dma_start(out=w8_0[:, s], in_=w8s[:, s])
            for kp in range(kbt0 // 2):
                s = slice(2 * kp, 2 * kp + 2)
                nc.sync.dma_start(out=x0[:, s], in_=xs[0, :, s])
                nc.sync.dma_start(out=w0[:, s], in_=ws[:, s])

            w_cur = (e0, w0, w8_0)
            x_cur = (0, x0, x8_0)
            for m in range(U):
                e = owner[m]
                k8t, kbt, x8s, xs, w8s, ws, cbase = cls_of(e)
                if w_cur[0] != e:
                    w8_t = w8pool.tile([P, k8t, N_CORE], f8, tag="w8")
                    nc.sync.dma_start(out=w8_t[:], in_=w8s[:])
                    w_t = wpool.tile([P, kbt, N_CORE], bf16, tag="w")
                    nc.sync.dma_start(out=w_t[:], in_=ws[:])
                    w_cur = (e, w_t, w8_t)
                _, w_t, w8_t = w_cur
                c, half = divmod(m, 2)
                if x_cur[0] != c:
                    x8_t = x8pool.tile([P, k8t, 2 * P], f8, tag="x8")
                    nc.sync.dma_start(out=x8_t[:], in_=x8s[c - cbase])
                    x_t = xpool.tile([P, kbt, 2 * P], bf16, tag="x")
                    nc.sync.dma_start(out=x_t[:], in_=xs[c - cbase])
                    x_cur = (c, x_t, x8_t)
                _, x_t, x8_t = x_cur

                hs = slice(half * P, (half + 1) * P)
                ps = pspool.tile([P, N_CORE], f32, tag="ps")
                # fp8 DoubleRow pairs, one full-width matmul per pair
                # (fp8 moving operand supports 128x1024).
                for j in range(k8t // 2):
                    js = slice(2 * j, 2 * j + 2)
                    nc.tensor.matmul(
                        ps[:],
                        x8_t[:, js, hs],
                        w8_t[:, js, :],
                        start=(j == 0),
                        stop=False,
                        perf_mode=DR,
                        skip_group_check=True,
                    )
                    if m == 0:
                        dummy(DR_FILL)
                for k in range(kbt):
                    nc.tensor.matmul(
                        ps[:],
                        x_t[:, k, hs],
                        w_t[:, k, :],
                        start=False,
                        stop=(k == kbt - 1),
                        skip_group_check=True,
                    )
                    if m == 0 and k % 2 == 1:
                        dummy(PAIR_FILL)
                o_t = opool.tile([P, N_CORE], bf16, tag="o")
                nc.vector.tensor_scalar_mul(o_t[:], ps[:], 1.0 / SCALE)
                nc.scalar.dma_start(out=o_p[m * P : (m + 1) * P, :], in_=o_t[:])
    nc.compile()
    return nc


def _get_program(units: tuple[int, ...], k8a: int):
    key = (units, k8a)
    if key not in _BUILD_CACHE:
        _BUILD_CACHE[key] = _build_program(units, k8a)
    return _BUILD_CACHE[key]


def _segments(tokens_per_expert: np.ndarray, total: int):
    sizes = []
    start = 0
    for e in range(GROUPS):
        s = int(max(0, tokens_per_expert[e]))
        s = min(s, total - start)
        sizes.append(s)
        start += s
    return sizes


def _pack(Xp, k8t, f8s, bfs):
    """Xp [R, K] fp32 -> (x8 [C, P, k8t, 2P] e4m3, xb [C, P, KT-k8t, 2P] bf16)."""
    R, K = Xp.shape
    C = R // (2 * P)
    K8 = k8t * P
    x8 = np.ascontiguousarray(
        (Xp[:, :K8] * f8s).astype(ml_dtypes.float8_e4m3)
        .reshape(C, 2 * P, k8t, P).transpose(0, 3, 2, 1)
    )
    xb = np.ascontiguousarray(
        (Xp[:, K8:] * bfs).astype(ml_dtypes.bfloat16)
        .reshape(C, 2 * P, KT - k8t, P).transpose(0, 3, 2, 1)
    )
    return x8, xb


def kernel(input, weight, tokens_per_expert, _trace=False, _trace_kwargs=None):
    from concourse.bass_utils import run_bass_kernel_spmd

    x = np.asarray(input, dtype=np.float32)
    w = np.asarray(weight, dtype=np.float32)
    tpe = np.asarray(tokens_per_expert, dtype=np.int64)
    T, K = x.shape
    G, K2, N = w.shape
    assert (T, K, G, K2, N) == (NUM_TOKENS, IN_FEATURES, GROUPS, IN_FEATURES, OUT_FEATURES)

    sizes = _segments(tpe, T)
    units = [-(-s // P) for s in sizes]
    if sum(units) % 2:
        for e in range(GROUPS):
            if units[e] > 0 or e == GROUPS - 1:
                units[e] += 1
                break
    out = np.zeros((T, N), dtype=np.float32)
    U = sum(units)
    if U == 0:
        return out

    proc = sorted(range(GROUPS), key=lambda e: (-units[e], e))
    units_proc = tuple(units[e] for e in proc)
    k8a = K8A if units_proc[0] % 2 == 0 and units_proc[0] > 0 else K8B
    mstarts_proc = np.concatenate([[0], np.cumsum(units_proc)])[:GROUPS]
    pos = [0] * GROUPS
    for i, e in enumerate(proc):
        pos[e] = i
    orig_starts = np.concatenate([[0], np.cumsum(sizes)])[:GROUPS]

    Xp = np.zeros((U * P, K), dtype=np.float32)
    for e in range(GROUPS):
        s = sizes[e]
        if s:
            r0 = mstarts_proc[pos[e]] * P
            Xp[r0 : r0 + s] = x[orig_starts[e] : orig_starts[e] + s]

    has_a = k8a != K8B
    UA = units_proc[0] if has_a else 0
    RA = UA * P
    if has_a:
        x8a, xa = _pack(Xp[:RA], k8a, SX, 1.0)
    else:
        x8a = np.zeros((1, P, k8a, 2 * P), ml_dtypes.float8_e4m3)
        xa = np.zeros((1, P, KT - k8a, 2 * P), ml_dtypes.bfloat16)
    if U > UA:
        x8b, xb = _pack(Xp[RA:], K8B, SX, 1.0)
    else:
        x8b = np.zeros((1, P, K8B, 2 * P), ml_dtypes.float8_e4m3)
        xb = np.zeros((1, P, KT - K8B, 2 * P), ml_dtypes.bfloat16)

    wproc = w[proc]  # [G, K, N] processing order
    K8a = k8a * P
    K8b = K8B * P

    def wpack(mat, k8, scale8, scaleb, cslice):
        w8 = np.ascontiguousarray(
            (mat[:k8 * P, cslice] * scale8).astype(ml_dtypes.float8_e4m3)
            .reshape(k8, P, -1).transpose(1, 0, 2)
        )
        wb = np.ascontiguousarray(
            (mat[k8 * P :, cslice] * scaleb).astype(ml_dtypes.bfloat16)
            .reshape(KT - k8, P, -1).transpose(1, 0, 2)
        )
        return w8, wb

    boff = 1 if has_a else 0
    GB = sum(1 for u in units_proc[boff:] if u > 0)
    in_maps = []
    for cidx in range(N_CORES):
        cs = slice(cidx * N_CORE, (cidx + 1) * N_CORE)
        w8a_d, wa_d = wpack(wproc[0], k8a, SW, SCALE, cs)
        if GB:
            pk = [wpack(wproc[boff + i], K8B, SW, SCALE, cs) for i in range(GB)]
            w8b_d = np.stack([p[0] for p in pk])
            wb_d = np.stack([p[1] for p in pk])
        else:
            w8b_d = np.zeros((1, P, K8B, N_CORE), ml_dtypes.float8_e4m3)
            wb_d = np.zeros((1, P, KT - K8B, N_CORE), ml_dtypes.bfloat16)
        in_maps.append(
            {"xa": xa, "x8a": x8a, "xb": xb, "x8b": x8b,
             "wa": wa_d, "w8a": w8a_d, "wb": wb_d, "w8b": w8b_d}
        )

    nc = _get_program(units_proc, k8a)
    kw = dict(_trace_kwargs or {})
    res = run_bass_kernel_spmd(nc, in_maps, list(range(N_CORES)), trace=_trace, **kw)
    full = np.concatenate(
        [np.asarray(res.results[c]["out"]).astype(np.float32) for c in range(N_CORES)],
        axis=1,
    )

    for e in range(GROUPS):
        s = sizes[e]
        if s:
            r0 = mstarts_proc[pos[e]] * P
            out[orig_starts[e] : orig_starts[e] + s] = full[r0 : r0 + s]
    if _trace:
        return out, res
    return out


# revision 10
# speedup vs baseline: 1.0088x; 1.0040x over previous
"""Grouped-experts GEMM (MoE ragged dot) on 8 TRN2 NeuronCores.

Sharding: tensor-parallel over out_features. Every core sees all tokens
(identical SPMD program) and computes a disjoint 512-wide slice of the
4096 output columns; gather is a host-side concatenate, no collectives.

Compute: mixed bf16 + fp8 (e4m3) with fp32 PSUM accumulation. A fraction
of the K=2048 contraction runs as fp8 DoubleRow pairs (2 fp8 MACs per PE
cell per cycle): the first-processed (largest) expert runs 12 of 16
K-tiles in fp8, the rest run 2 of 16. The 2e-2 rel-err budget is spent
where it buys the most compute (measured HW rel err 1.863e-2, matching
the host e4m3 simulation exactly). fp8 operands are pre-scaled by
SX=8/SW=128 to clear the e4m3 subnormal region; the bf16 W carries the
combined 1024x so one exact tensor_scalar multiply (1/1024) in the
output copy undoes it. Outputs store as bf16 (host upcasts).

Schedule (trace-driven):
 - Experts processed in descending padded-tile order, so each later 2MB
   W load always trails the X stream with slack.
 - Warm-up dummy matmuls before and DMA-paced inside m-tile 0 keep the
   PE busy through the DMA-bound prologue, holding the HAM clock gate at
   K=8/8 (2.4 GHz) instead of idling cold at 1.2 GHz.
 - The first expert's W + first X chunk load as interleaved k-pair
   pieces so the first matmul waits on ~0.4MB, not 3MB.
 - start=True on a sub-bank matmul clears has_written for the WHOLE
   PSUM bank on HW: only the first matmul of each tile's group sets it.

fp8 pairs run as ONE full-width DoubleRow matmul each (fp8 moving
operand supports 128x1024): ~228ns/pair vs 309ns for two 256-wide halves.

Measured: ~219us HW exec (baseline bf16 kernel: ~236us); bf16 floor
for this shape would be ~221us of pure matmul issue alone.

Fallback: if the first expert's padded tile count is odd (chunks would
straddle the fp8-class boundary), everything runs k8=2 uniformly.
"""

import sys

import numpy as np

sys.path.insert(0, "/opt/trn_rl_repo")

import ml_dtypes

NUM_TOKENS = 8192
IN_FEATURES = 2048
OUT_FEATURES = 4096
GROUPS = 8
N_CORES = 8

P = 128
KT = IN_FEATURES // P  # 16 K-tiles of 128
N_CORE = OUT_FEATURES // N_CORES  # 512 output cols per core

K8A = 12  # fp8 K-tiles for the first (largest) expert
K8B = 2  # fp8 K-tiles for the rest

SX = 8.0  # fp8 x pre-scale
SW = 128.0  # fp8 w pre-scale
SCALE = SX * SW  # PSUM carries SCALE x true output
PRE_WARM = 8
DR_FILL = 2
PAIR_FILL = 3

_BUILD_CACHE: dict = {}


def _build_program(units: tuple[int, ...], k8a: int):
    """SPMD program for per-position padded M-tile counts `units`
    (processing order). Position 0 runs k8a fp8 K-tiles, rest K8B."""
    import concourse.mybir as mybir
    import concourse.tile as tile
    from concourse import bacc

    f32 = mybir.dt.float32
    bf16 = mybir.dt.bfloat16
    f8 = mybir.dt.float8e4
    DR = mybir.MatmulPerfMode.DoubleRow
    U = sum(units)
    assert U % 2 == 0
    has_a = k8a != K8B  # no class A in the fallback (odd first-expert units)
    UA = units[0] if has_a else 0
    assert UA % 2 == 0
    CA = UA // 2  # class-A chunks
    CB = (U - UA) // 2
    KBA, KBB = KT - k8a, KT - K8B
    GB = sum(1 for u in units[1 if has_a else 0 :] if u > 0)

    nc = bacc.Bacc(None, target_bir_lowering=False)
    dp = nc.declare_dram_parameter
    xa_p = dp("xa", [max(CA, 1), P, KBA, 2 * P], bf16, isOutput=False)
    x8a_p = dp("x8a", [max(CA, 1), P, k8a, 2 * P], f8, isOutput=False)
    wa_p = dp("wa", [P, KBA, N_CORE], bf16, isOutput=False)
    w8a_p = dp("w8a", [P, k8a, N_CORE], f8, isOutput=False)
    xb_p = dp("xb", [max(CB, 1), P, KBB, 2 * P], bf16, isOutput=False)
    x8b_p = dp("x8b", [max(CB, 1), P, K8B, 2 * P], f8, isOutput=False)
    wb_p = dp("wb", [max(GB, 1), P, KBB, N_CORE], bf16, isOutput=False)
    w8b_p = dp("w8b", [max(GB, 1), P, K8B, N_CORE], f8, isOutput=False)
    o_p = dp("out", [U * P, N_CORE], bf16, isOutput=True)

    with tile.TileContext(nc) as tc:
        with (
            tc.tile_pool(name="warm", bufs=1) as warmpool,
            tc.tile_pool(name="xp", bufs=5) as xpool,
            tc.tile_pool(name="x8p", bufs=5) as x8pool,
            tc.tile_pool(name="wp", bufs=3) as wpool,
            tc.tile_pool(name="w8p", bufs=3) as w8pool,
            tc.tile_pool(name="op", bufs=4) as opool,
            tc.tile_pool(name="ps", bufs=4, space="PSUM") as pspool,
        ):
            owner = []
            for e in range(len(units)):
                owner += [e] * units[e]

            scratch = warmpool.tile([P, P + N_CORE], bf16, tag="warm")
            nc.vector.memset(scratch[:], 0.0)
            wps = pspool.tile([P, N_CORE], f32, tag="ps")

            def dummy(n):
                for _ in range(n):
                    nc.tensor.matmul(
                        wps[:], scratch[:, :P], scratch[:, P:],
                        start=True, stop=True, skip_group_check=True,
                    )

            dummy(PRE_WARM)

            def cls_of(e):  # (k8t, kbt, x8_p, x_p, w8_ap, w_ap, cbase)
                if e == 0 and has_a:
                    return (k8a, KBA, x8a_p, xa_p, w8a_p, wa_p, 0)
                eb = e - 1 if has_a else e
                return (K8B, KBB, x8b_p, xb_p, w8b_p[eb], wb_p[eb], CA)

            # Prologue: first expert's W + first chunk, k-pair pieces,
            # fp8 pairs first.
            e0 = owner[0]
            k8t0, kbt0, x8s, xs, w8s, ws, _ = cls_of(e0)
            w8_0 = w8pool.tile([P, k8t0, N_CORE], f8, tag="w8")
            x8_0 = x8pool.tile([P, k8t0, 2 * P], f8, tag="x8")
            w0 = wpool.tile([P, kbt0, N_CORE], bf16, tag="w")
            x0 = xpool.tile([P, kbt0, 2 * P], bf16, tag="x")
            # 4-ktile pieces: each dma_start costs ~590ns of queue push
            # time, so fewer/larger pieces keep the prologue transfer-bound
            # instead of push-bound.
            for kp in range(0, k8t0, 4):
                s = slice(kp, min(kp + 4, k8t0))
                nc.sync.dma_start(out=x8_0[:, s], in_=x8s[0, :, s])
                nc.sync.dma_start(out=w8_0[:, s], in_=w8s[:, s])
            for kp in range(0, kbt0, 4):
                s = slice(kp, min(kp + 4, kbt0))
                nc.sync.dma_start(out=x0[:, s], in_=xs[0, :, s])
                nc.sync.dma_start(out=w0[:, s], in_=ws[:, s])

            w_cur = (e0, w0, w8_0)
            x_cur = (0, x0, x8_0)
            for m in range(U):
                e = owner[m]
                k8t, kbt, x8s, xs, w8s, ws, cbase = cls_of(e)
                if w_cur[0] != e:
                    w8_t = w8pool.tile([P, k8t, N_CORE], f8, tag="w8")
                    nc.sync.dma_start(out=w8_t[:], in_=w8s[:])
                    w_t = wpool.tile([P, kbt, N_CORE], bf16, tag="w")
                    nc.sync.dma_start(out=w_t[:], in_=ws[:])
                    w_cur = (e, w_t, w8_t)
                _, w_t, w8_t = w_cur
                c, half = divmod(m, 2)
                if x_cur[0] != c:
                    x8_t = x8pool.tile([P, k8t, 2 * P], f8, tag="x8")
                    nc.sync.dma_start(out=x8_t[:], in_=x8s[c - cbase])
                    x_t = xpool.tile([P, kbt, 2 * P], bf16, tag="x")
                    nc.sync.dma_start(out=x_t[:], in_=xs[c - cbase])
                    x_cur = (c, x_t, x8_t)
                _, x_t, x8_t = x_cur

                hs = slice(half * P, (half + 1) * P)
                ps = pspool.tile([P, N_CORE], f32, tag="ps")
                # fp8 DoubleRow pairs, one full-width matmul per pair
                # (fp8 moving operand supports 128x1024).
                for j in range(k8t // 2):
                    js = slice(2 * j, 2 * j + 2)
                    nc.tensor.matmul(
                        ps[:],
                        x8_t[:, js, hs],
                        w8_t[:, js, :],
                        start=(j == 0),
                        stop=False,
                        perf_mode=DR,
                        skip_group_check=True,
                    )
                    if m == 0:
                        dummy(DR_FILL)
                for k in range(kbt):
                    nc.tensor.matmul(
                        ps[:],
                        x_t[:, k, hs],
                        w_t[:, k, :],
                        start=False,
                        stop=(k == kbt - 1),
                        skip_group_check=True,
                    )
                    if m == 0 and k % 2 == 1:
                        dummy(PAIR_FILL)
                o_t = opool.tile([P, N_CORE], bf16, tag="o")
                nc.vector.tensor_scalar_mul(o_t[:], ps[:], 1.0 / SCALE)
                nc.scalar.dma_start(out=o_p[m * P : (m + 1) * P, :], in_=o_t[:])
    nc.compile()
    return nc


def _get_program(units: tuple[int, ...], k8a: int):
    key = (units, k8a)
    if key not in _BUILD_CACHE:
        _BUILD_CACHE[key] = _build_program(units, k8a)
    return _BUILD_CACHE[key]


def _segments(tokens_per_expert: np.ndarray, total: int):
    sizes = []
    start = 0
    for e in range(GROUPS):
        s = int(max(0, tokens_per_expert[e]))
        s = min(s, total - start)
        sizes.append(s)
        start += s
    return sizes


def _pack(Xp, k8t, f8s, bfs):
    """Xp [R, K] fp32 -> (x8 [C, P, k8t, 2P] e4m3, xb [C, P, KT-k8t, 2P] bf16)."""
    R, K = Xp.shape
    C = R // (2 * P)
    K8 = k8t * P
    x8 = np.ascontiguousarray(
        (Xp[:, :K8] * f8s).astype(ml_dtypes.float8_e4m3)
        .reshape(C, 2 * P, k8t, P).transpose(0, 3, 2, 1)
    )
    xb = np.ascontiguousarray(
        (Xp[:, K8:] * bfs).astype(ml_dtypes.bfloat16)
        .reshape(C, 2 * P, KT - k8t, P).transpose(0, 3, 2, 1)
    )
    return x8, xb


def kernel(input, weight, tokens_per_expert, _trace=False, _trace_kwargs=None):
    from concourse.bass_utils import run_bass_kernel_spmd

    x = np.asarray(input, dtype=np.float32)
    w = np.asarray(weight, dtype=np.float32)
    tpe = np.asarray(tokens_per_expert, dtype=np.int64)
    T, K = x.shape
    G, K2, N = w.shape
    assert (T, K, G, K2, N) == (NUM_TOKENS, IN_FEATURES, GROUPS, IN_FEATURES, OUT_FEATURES)

    sizes = _segments(tpe, T)
    units = [-(-s // P) for s in sizes]
    if sum(units) % 2:
        for e in range(GROUPS):
            if units[e] > 0 or e == GROUPS - 1:
                units[e] += 1
                break
    out = np.zeros((T, N), dtype=np.float32)
    U = sum(units)
    if U == 0:
        return out

    proc = sorted(range(GROUPS), key=lambda e: (-units[e], e))
    units_proc = tuple(units[e] for e in proc)
    k8a = K8A if units_proc[0] % 2 == 0 and units_proc[0] > 0 else K8B
    mstarts_proc = np.concatenate([[0], np.cumsum(units_proc)])[:GROUPS]
    pos = [0] * GROUPS
    for i, e in enumerate(proc):
        pos[e] = i
    orig_starts = np.concatenate([[0], np.cumsum(sizes)])[:GROUPS]

    Xp = np.zeros((U * P, K), dtype=np.float32)
    for e in range(GROUPS):
        s = sizes[e]
        if s:
            r0 = mstarts_proc[pos[e]] * P
            Xp[r0 : r0 + s] = x[orig_starts[e] : orig_starts[e] + s]

    has_a = k8a != K8B
    UA = units_proc[0] if has_a else 0
    RA = UA * P
    if has_a:
        x8a, xa = _pack(Xp[:RA], k8a, SX, 1.0)
    else:
        x8a = np.zeros((1, P, k8a, 2 * P), ml_dtypes.float8_e4m3)
        xa = np.zeros((1, P, KT - k8a, 2 * P), ml_dtypes.bfloat16)
    if U > UA:
        x8b, xb = _pack(Xp[RA:], K8B, SX, 1.0)
    else:
        x8b = np.zeros((1, P, K8B, 2 * P), ml_dtypes.float8_e4m3)
        xb = np.zeros((1, P, KT - K8B, 2 * P), ml_dtypes.bfloat16)

    wproc = w[proc]  # [G, K, N] processing order
    K8a = k8a * P
    K8b = K8B * P

    def wpack(mat, k8, scale8, scaleb, cslice):
        w8 = np.ascontiguousarray(
            (mat[:k8 * P, cslice] * scale8).astype(ml_dtypes.float8_e4m3)
            .reshape(k8, P, -1).transpose(1, 0, 2)
        )
        wb = np.ascontiguousarray(
            (mat[k8 * P :, cslice] * scaleb).astype(ml_dtypes.bfloat16)
            .reshape(KT - k8, P, -1).transpose(1, 0, 2)
        )
        return w8, wb

    boff = 1 if has_a else 0
    GB = sum(1 for u in units_proc[boff:] if u > 0)
    in_maps = []
    for cidx in range(N_CORES):
        cs = slice(cidx * N_CORE, (cidx + 1) * N_CORE)
        w8a_d, wa_d = wpack(wproc[0], k8a, SW, SCALE, cs)
        if GB:
            pk = [wpack(wproc[boff + i], K8B, SW, SCALE, cs) for i in range(GB)]
            w8b_d = np.stack([p[0] for p in pk])
            wb_d = np.stack([p[1] for p in pk])
        else:
            w8b_d = np.zeros((1, P, K8B, N_CORE), ml_dtypes.float8_e4m3)
            wb_d = np.zeros((1, P, KT - K8B, N_CORE), ml_dtypes.bfloat16)
        in_maps.append(
            {"xa": xa, "x8a": x8a, "xb": xb, "x8b": x8b,
             "wa": wa_d, "w8a": w8a_d, "wb": wb_d, "w8b": w8b_d}
        )

    nc = _get_program(units_proc, k8a)
    kw = dict(_trace_kwargs or {})
    res = run_bass_kernel_spmd(nc, in_maps, list(range(N_CORES)), trace=_trace, **kw)
    full = np.concatenate(
        [np.asarray(res.results[c]["out"]).astype(np.float32) for c in range(N_CORES)],
        axis=1,
    )

    for e in range(GROUPS):
        s = sizes[e]
        if s:
            r0 = mstarts_proc[pos[e]] * P
            out[orig_starts[e] : orig_starts[e] + s] = full[r0 : r0 + s]
    if _trace:
        return out, res
    return out


# revision 11
# speedup vs baseline: 1.0198x; 1.0109x over previous
"""Grouped-experts GEMM (MoE ragged dot) on 8 TRN2 NeuronCores.

Sharding: tensor-parallel over out_features. Every core sees all tokens
(identical SPMD program) and computes a disjoint 512-wide slice of the
4096 output columns; gather is a host-side concatenate, no collectives.

Compute: mixed bf16 + fp8 (e4m3) with fp32 PSUM accumulation. A fraction
of the K=2048 contraction runs as fp8 DoubleRow pairs (2 fp8 MACs per PE
cell per cycle): the first-processed (largest) expert runs 12 of 16
K-tiles in fp8, the rest run 2 of 16. The 2e-2 rel-err budget is spent
where it buys the most compute (measured HW rel err 1.863e-2, matching
the host e4m3 simulation exactly). fp8 operands are pre-scaled by
SX=8/SW=128 to clear the e4m3 subnormal region; the bf16 W carries the
combined 1024x so one exact tensor_scalar multiply (1/1024) in the
output copy undoes it. Outputs store as bf16 (host upcasts).

Schedule (trace-driven):
 - Experts processed in descending padded-tile order, so each later 2MB
   W load always trails the X stream with slack.
 - Warm-up dummy matmuls before and DMA-paced inside m-tile 0 keep the
   PE busy through the DMA-bound prologue, holding the HAM clock gate at
   K=8/8 (2.4 GHz) instead of idling cold at 1.2 GHz.
 - The first expert's W + first X chunk load as interleaved k-pair
   pieces so the first matmul waits on ~0.4MB, not 3MB.
 - start=True on a sub-bank matmul clears has_written for the WHOLE
   PSUM bank on HW: only the first matmul of each tile's group sets it.

fp8 pairs run as ONE full-width DoubleRow matmul each (fp8 moving
operand supports 128x1024): ~228ns/pair vs 309ns for two 256-wide halves.

Measured: ~219us HW exec (baseline bf16 kernel: ~236us); bf16 floor
for this shape would be ~221us of pure matmul issue alone.

Fallback: if the first expert's padded tile count is odd (chunks would
straddle the fp8-class boundary), everything runs k8=2 uniformly.
"""

import sys

import numpy as np

sys.path.insert(0, "/opt/trn_rl_repo")

import ml_dtypes

NUM_TOKENS = 8192
IN_FEATURES = 2048
OUT_FEATURES = 4096
GROUPS = 8
N_CORES = 8

P = 128
KT = IN_FEATURES // P  # 16 K-tiles of 128
N_CORE = OUT_FEATURES // N_CORES  # 512 output cols per core

K8A = 12  # fp8 K-tiles for the first (largest) expert
K8B = 2  # fp8 K-tiles for the rest

SX = 8.0  # fp8 x pre-scale
SW = 128.0  # fp8 w pre-scale
SCALE = SX * SW  # PSUM carries SCALE x true output
PRE_WARM = 8
DR_FILL = 2
PAIR_FILL = 3

_BUILD_CACHE: dict = {}


def _build_program(units: tuple[int, ...], k8a: int):
    """SPMD program for per-position padded M-tile counts `units`
    (processing order). Position 0 runs k8a fp8 K-tiles, rest K8B."""
    import concourse.mybir as mybir
    import concourse.tile as tile
    from concourse import bacc

    f32 = mybir.dt.float32
    bf16 = mybir.dt.bfloat16
    f8 = mybir.dt.float8e4
    DR = mybir.MatmulPerfMode.DoubleRow
    U = sum(units)
    assert U % 2 == 0
    has_a = k8a != K8B  # no class A in the fallback (odd first-expert units)
    UA = units[0] if has_a else 0
    assert UA % 2 == 0
    CA = UA // 2  # class-A chunks
    CB = (U - UA) // 2
    KBA, KBB = KT - k8a, KT - K8B
    GB = sum(1 for u in units[1 if has_a else 0 :] if u > 0)

    nc = bacc.Bacc(None, target_bir_lowering=False)
    dp = nc.declare_dram_parameter
    xa_p = dp("xa", [max(CA, 1), P, KBA, 2 * P], bf16, isOutput=False)
    x8a_p = dp("x8a", [max(CA, 1), P, k8a, 2 * P], f8, isOutput=False)
    wa_p = dp("wa", [P, KBA, N_CORE], bf16, isOutput=False)
    w8a_p = dp("w8a", [P, k8a, N_CORE], f8, isOutput=False)
    xb_p = dp("xb", [max(CB, 1), P, KBB, 2 * P], bf16, isOutput=False)
    x8b_p = dp("x8b", [max(CB, 1), P, K8B, 2 * P], f8, isOutput=False)
    wb_p = dp("wb", [max(GB, 1), P, KBB, N_CORE], bf16, isOutput=False)
    w8b_p = dp("w8b", [max(GB, 1), P, K8B, N_CORE], f8, isOutput=False)
    o_p = dp("out", [U * P, N_CORE], bf16, isOutput=True)

    with tile.TileContext(nc) as tc:
        with (
            tc.tile_pool(name="warm", bufs=1) as warmpool,
            tc.tile_pool(name="xp", bufs=5) as xpool,
            tc.tile_pool(name="x8p", bufs=5) as x8pool,
            tc.tile_pool(name="wp", bufs=3) as wpool,
            tc.tile_pool(name="w8p", bufs=3) as w8pool,
            tc.tile_pool(name="op", bufs=4) as opool,
            tc.tile_pool(name="ps", bufs=4, space="PSUM") as pspool,
        ):
            owner = []
            for e in range(len(units)):
                owner += [e] * units[e]

            scratch = warmpool.tile([P, P + N_CORE], bf16, tag="warm")
            nc.vector.memset(scratch[:], 0.0)
            wps = pspool.tile([P, N_CORE], f32, tag="ps")

            def dummy(n):
                for _ in range(n):
                    nc.tensor.matmul(
                        wps[:], scratch[:, :P], scratch[:, P:],
                        start=True, stop=True, skip_group_check=True,
                    )

            dummy(PRE_WARM)

            def cls_of(e):  # (k8t, kbt, x8_p, x_p, w8_ap, w_ap, cbase)
                if e == 0 and has_a:
                    return (k8a, KBA, x8a_p, xa_p, w8a_p, wa_p, 0)
                eb = e - 1 if has_a else e
                return (K8B, KBB, x8b_p, xb_p, w8b_p[eb], wb_p[eb], CA)

            # Prologue: first expert's W + first chunk, k-pair pieces,
            # fp8 pairs first.
            e0 = owner[0]
            k8t0, kbt0, x8s, xs, w8s, ws, _ = cls_of(e0)
            w8_0 = w8pool.tile([P, k8t0, N_CORE], f8, tag="w8")
            x8_0 = x8pool.tile([P, k8t0, 2 * P], f8, tag="x8")
            w0 = wpool.tile([P, kbt0, N_CORE], bf16, tag="w")
            x0 = xpool.tile([P, kbt0, 2 * P], bf16, tag="x")
            # 4-ktile pieces: each dma_start costs ~590ns of queue push
            # time, so fewer/larger pieces keep the prologue transfer-bound
            # instead of push-bound.
            for kp in range(0, k8t0, 4):
                s = slice(kp, min(kp + 4, k8t0))
                nc.sync.dma_start(out=x8_0[:, s], in_=x8s[0, :, s])
                nc.sync.dma_start(out=w8_0[:, s], in_=w8s[:, s])
            for kp in range(0, kbt0, 4):
                s = slice(kp, min(kp + 4, kbt0))
                nc.sync.dma_start(out=x0[:, s], in_=xs[0, :, s])
                nc.sync.dma_start(out=w0[:, s], in_=ws[:, s])

            w_cur = (e0, w0, w8_0)
            x_cur = (0, x0, x8_0)
            for m in range(U):
                e = owner[m]
                k8t, kbt, x8s, xs, w8s, ws, cbase = cls_of(e)
                c, half = divmod(m, 2)
                # X chunk before W at expert crossings: the 2MB W load has
                # slack, the next chunks do not.
                if x_cur[0] != c:
                    x8_t = x8pool.tile([P, k8t, 2 * P], f8, tag="x8")
                    nc.sync.dma_start(out=x8_t[:], in_=x8s[c - cbase])
                    x_t = xpool.tile([P, kbt, 2 * P], bf16, tag="x")
                    nc.sync.dma_start(out=x_t[:], in_=xs[c - cbase])
                    x_cur = (c, x_t, x8_t)
                _, x_t, x8_t = x_cur
                if w_cur[0] != e:
                    w8_t = w8pool.tile([P, k8t, N_CORE], f8, tag="w8")
                    nc.sync.dma_start(out=w8_t[:], in_=w8s[:])
                    w_t = wpool.tile([P, kbt, N_CORE], bf16, tag="w")
                    nc.sync.dma_start(out=w_t[:], in_=ws[:])
                    w_cur = (e, w_t, w8_t)
                _, w_t, w8_t = w_cur

                hs = slice(half * P, (half + 1) * P)
                ps = pspool.tile([P, N_CORE], f32, tag="ps")
                # fp8 DoubleRow pairs, one full-width matmul per pair
                # (fp8 moving operand supports 128x1024).
                for j in range(k8t // 2):
                    js = slice(2 * j, 2 * j + 2)
                    nc.tensor.matmul(
                        ps[:],
                        x8_t[:, js, hs],
                        w8_t[:, js, :],
                        start=(j == 0),
                        stop=False,
                        perf_mode=DR,
                        skip_group_check=True,
                    )
                    if m == 0:
                        dummy(DR_FILL)
                for k in range(kbt):
                    nc.tensor.matmul(
                        ps[:],
                        x_t[:, k, hs],
                        w_t[:, k, :],
                        start=False,
                        stop=(k == kbt - 1),
                        skip_group_check=True,
                    )
                    if m == 0 and k % 2 == 1:
                        dummy(PAIR_FILL)
                o_t = opool.tile([P, N_CORE], bf16, tag="o")
                nc.vector.tensor_scalar_mul(o_t[:], ps[:], 1.0 / SCALE)
                nc.scalar.dma_start(out=o_p[m * P : (m + 1) * P, :], in_=o_t[:])
    nc.compile()
    return nc


def _get_program(units: tuple[int, ...], k8a: int):
    key = (units, k8a)
    if key not in _BUILD_CACHE:
        _BUILD_CACHE[key] = _build_program(units, k8a)
    return _BUILD_CACHE[key]


def _segments(tokens_per_expert: np.ndarray, total: int):
    sizes = []
    start = 0
    for e in range(GROUPS):
        s = int(max(0, tokens_per_expert[e]))
        s = min(s, total - start)
        sizes.append(s)
        start += s
    return sizes


def _pack(Xp, k8t, f8s, bfs):
    """Xp [R, K] fp32 -> (x8 [C, P, k8t, 2P] e4m3, xb [C, P, KT-k8t, 2P] bf16)."""
    R, K = Xp.shape
    C = R // (2 * P)
    K8 = k8t * P
    x8 = np.ascontiguousarray(
        (Xp[:, :K8] * f8s).astype(ml_dtypes.float8_e4m3)
        .reshape(C, 2 * P, k8t, P).transpose(0, 3, 2, 1)
    )
    xb = np.ascontiguousarray(
        (Xp[:, K8:] * bfs).astype(ml_dtypes.bfloat16)
        .reshape(C, 2 * P, KT - k8t, P).transpose(0, 3, 2, 1)
    )
    return x8, xb


def kernel(input, weight, tokens_per_expert, _trace=False, _trace_kwargs=None):
    from concourse.bass_utils import run_bass_kernel_spmd

    x = np.asarray(input, dtype=np.float32)
    w = np.asarray(weight, dtype=np.float32)
    tpe = np.asarray(tokens_per_expert, dtype=np.int64)
    T, K = x.shape
    G, K2, N = w.shape
    assert (T, K, G, K2, N) == (NUM_TOKENS, IN_FEATURES, GROUPS, IN_FEATURES, OUT_FEATURES)

    sizes = _segments(tpe, T)
    units = [-(-s // P) for s in sizes]
    if sum(units) % 2:
        for e in range(GROUPS):
            if units[e] > 0 or e == GROUPS - 1:
                units[e] += 1
                break
    out = np.zeros((T, N), dtype=np.float32)
    U = sum(units)
    if U == 0:
        return out

    proc = sorted(range(GROUPS), key=lambda e: (-units[e], e))
    units_proc = tuple(units[e] for e in proc)
    k8a = K8A if units_proc[0] % 2 == 0 and units_proc[0] > 0 else K8B
    mstarts_proc = np.concatenate([[0], np.cumsum(units_proc)])[:GROUPS]
    pos = [0] * GROUPS
    for i, e in enumerate(proc):
        pos[e] = i
    orig_starts = np.concatenate([[0], np.cumsum(sizes)])[:GROUPS]

    Xp = np.zeros((U * P, K), dtype=np.float32)
    for e in range(GROUPS):
        s = sizes[e]
        if s:
            r0 = mstarts_proc[pos[e]] * P
            Xp[r0 : r0 + s] = x[orig_starts[e] : orig_starts[e] + s]

    has_a = k8a != K8B
    UA = units_proc[0] if has_a else 0
    RA = UA * P
    if has_a:
        x8a, xa = _pack(Xp[:RA], k8a, SX, 1.0)
    else:
        x8a = np.zeros((1, P, k8a, 2 * P), ml_dtypes.float8_e4m3)
        xa = np.zeros((1, P, KT - k8a, 2 * P), ml_dtypes.bfloat16)
    if U > UA:
        x8b, xb = _pack(Xp[RA:], K8B, SX, 1.0)
    else:
        x8b = np.zeros((1, P, K8B, 2 * P), ml_dtypes.float8_e4m3)
        xb = np.zeros((1, P, KT - K8B, 2 * P), ml_dtypes.bfloat16)

    wproc = w[proc]  # [G, K, N] processing order
    K8a = k8a * P
    K8b = K8B * P

    def wpack(mat, k8, scale8, scaleb, cslice):
        w8 = np.ascontiguousarray(
            (mat[:k8 * P, cslice] * scale8).astype(ml_dtypes.float8_e4m3)
            .reshape(k8, P, -1).transpose(1, 0, 2)
        )
        wb = np.ascontiguousarray(
            (mat[k8 * P :, cslice] * scaleb).astype(ml_dtypes.bfloat16)
            .reshape(KT - k8, P, -1).transpose(1, 0, 2)
        )
        return w8, wb

    boff = 1 if has_a else 0
    GB = sum(1 for u in units_proc[boff:] if u > 0)
    in_maps = []
    for cidx in range(N_CORES):
        cs = slice(cidx * N_CORE, (cidx + 1) * N_CORE)
        w8a_d, wa_d = wpack(wproc[0], k8a, SW, SCALE, cs)
        if GB:
            pk = [wpack(wproc[boff + i], K8B, SW, SCALE, cs) for i in range(GB)]
            w8b_d = np.stack([p[0] for p in pk])
            wb_d = np.stack([p[1] for p in pk])
        else:
            w8b_d = np.zeros((1, P, K8B, N_CORE), ml_dtypes.float8_e4m3)
            wb_d = np.zeros((1, P, KT - K8B, N_CORE), ml_dtypes.bfloat16)
        in_maps.append(
            {"xa": xa, "x8a": x8a, "xb": xb, "x8b": x8b,
             "wa": wa_d, "w8a": w8a_d, "wb": wb_d, "w8b": w8b_d}
        )

    nc = _get_program(units_proc, k8a)
    kw = dict(_trace_kwargs or {})
    res = run_bass_kernel_spmd(nc, in_maps, list(range(N_CORES)), trace=_trace, **kw)
    full = np.concatenate(
        [np.asarray(res.results[c]["out"]).astype(np.float32) for c in range(N_CORES)],
        axis=1,
    )

    for e in range(GROUPS):
        s = sizes[e]
        if s:
            r0 = mstarts_proc[pos[e]] * P
            out[orig_starts[e] : orig_starts[e] + s] = full[r0 : r0 + s]
    if _trace:
        return out, res
    return out
